# revision 22
# baseline (speedup 1.0000x reference)
"""Trainium2 Bass kernel for nn_AsMac: soft-NW motif embedding distance matrix.

Pipeline per core (batch-sharded, 2 sequences/core, all 256 channels):
  1. conv[b,c,t] via PE matmul on im2col (contraction 320 = K*D), fp32r
  2. token stream via tiny matmul row (one-hot dot [0..4])
  3. argmax_t conv -> jmin  (DVE max/max_index)
  4. gather token windows (dma_gather, aligned expanded rows)
  5. gather s-matrix row PAIRS straight from a host-prebuilt W-pair table
     (s[i,:] = W[c, tok_i, :] because seq is one-hot) -- no per-problem matmuls
  6. hard-NW DP (gamma=0.01 makes soft~hard; validated 4.7e-4 final rel err)
     row recurrence as ONE scalar_tensor_tensor + 4 tensor_tensor_scan per row
     over 512 problems [128 partitions x 4 blocks], in a shifted variable
     Z[i,j] = D/gap + i + j so all boundaries are 0 and no per-row constants
  7. embed = relu(gap*(Z[64,64]-128) + bias); AllGather embeds; Gram on PE;
     normalize via Gram diagonal rsqrt (Newton-refined); out = 1 - cos.
"""

import sys

for _p in ("/opt/trn_rl_repo",):
    if _p not in sys.path:
        sys.path.insert(0, _p)

import numpy as np
from contextlib import ExitStack

import concourse.bass as bass
import concourse.bacc as bacc
import concourse.tile as tile
from concourse import mybir
from concourse.bass_utils import run_bass_kernel_spmd

F32 = mybir.dt.float32
I16 = mybir.dt.int16
U16 = mybir.dt.uint16
ALU = mybir.AluOpType

B, D, L, C, K = 16, 5, 1024, 256, 64
NCORES = 8
BPC = B // NCORES            # 2 sequences per core
T = L - K + 1                # 961 conv positions
KD = K * D                   # 320 contraction rows (k-major, d-minor)
NQ = 4                       # blocks q = b_lo*2 + c_hi
NPAIR = K // 2               # 32 row-pairs per problem
NPROB = 128 * NQ             # 512 problems per core
KCH = [(0, 25), (25, 25), (50, 14)]      # k-chunks -> 125/125/70 contraction rows
NCH = [(0, 512), (512, 449)]             # conv N chunks (<=512 moving free dim)

_GRAPH = None


def _ap(handle_ap, offset, dims):
    """Raw AP over the tensor underlying `handle_ap` (an AP or handle)."""
    t = handle_ap.tensor if isinstance(handle_ap, bass.AP) else handle_ap
    return bass.AP(t, offset, [list(d) for d in dims])


def _build():
    nc = bacc.Bacc(num_devices=NCORES)

    seq = nc.declare_dram_parameter("seq", [BPC, D, L], F32, isOutput=False)
    w2t = nc.declare_dram_parameter("w2t", [KD, C], F32, isOutput=False)
    wtok = nc.declare_dram_parameter("wtok", [128, 2], F32, isOutput=False)
    wpair = nc.declare_dram_parameter("wpair", [C * 25, 2 * K], F32, isOutput=False)
    c25 = nc.declare_dram_parameter("c25", [16, 8 * NQ * NPAIR], F32, isOutput=False)
    bias4 = nc.declare_dram_parameter("bias4", [128, NQ], F32, isOutput=False)
    gap4 = nc.declare_dram_parameter("gap4", [128, NQ], F32, isOutput=False)
    ident = nc.declare_dram_parameter("ident16", [B, B], F32, isOutput=False)
    outx = nc.declare_dram_parameter("out", [B, B], F32, isOutput=True)
    dbg_jm = nc.declare_dram_parameter("dbg_jm", [128, NQ], F32, isOutput=True)
    dbg_tokw = nc.declare_dram_parameter("dbg_tokw", [16, 32 * K], F32, isOutput=True)
    dbg_pix = nc.declare_dram_parameter("dbg_pix", [16, 32 * NPAIR], F32, isOutput=True)
    dbg_s = nc.declare_dram_parameter("dbg_s", [128, NQ * 2 * K], F32, isOutput=True)
    dbg_sc = nc.declare_dram_parameter("dbg_sc", [128, NQ], F32, isOutput=True)
    dbg_emb = nc.declare_dram_parameter("dbg_emb", [B, C], F32, isOutput=True)
    dbg_tok = nc.declare_dram_parameter("dbg_tok", [BPC, L], F32, isOutput=True)

    tokd = nc.dram_tensor("tokd", [BPC, L], F32)
    tokwin = nc.dram_tensor("tokwin", [BPC * L, K], F32)
    jmd = nc.dram_tensor("jmd", [NPROB], F32)
    embd = nc.dram_tensor("embd", [BPC, C], F32)
    emba = nc.dram_tensor("emba", [B, C], F32, addr_space="Shared")

    with tile.TileContext(nc) as tc, ExitStack() as ctx:
        cpool = ctx.enter_context(tc.tile_pool(name="consts", bufs=1))
        work = ctx.enter_context(tc.tile_pool(name="work", bufs=1))
        spool = ctx.enter_context(tc.tile_pool(name="small", bufs=2))
        psum = ctx.enter_context(tc.tile_pool(name="psum", bufs=2, space="PSUM"))
        tokps = ctx.enter_context(tc.tile_pool(name="tokps", bufs=1, space="PSUM"))
        gpsum = ctx.enter_context(tc.tile_pool(name="gpsum", bufs=1, space="PSUM"))

        # ---- constants to SBUF ----
        w2t_sb = []
        for ci, (k0, nk) in enumerate(KCH):
            t = cpool.tile([nk * 5, C], F32, tag=f"w2t{ci}")
            nc.sync.dma_start(out=t[:, :], in_=w2t[k0 * 5:(k0 + nk) * 5, :])
            w2t_sb.append(t)
        wtok_sb = cpool.tile([128, 2], F32, tag="wtok")
        nc.sync.dma_start(out=wtok_sb[:, :], in_=wtok[:, :])
        c25_sb = cpool.tile([16, 8 * NQ * NPAIR], F32, tag="c25")
        nc.sync.dma_start(out=c25_sb[:, :], in_=c25[:, :])
        bias_sb = cpool.tile([128, NQ], F32, tag="bias4")
        nc.sync.dma_start(out=bias_sb[:, :], in_=bias4[:, :])
        gap_sb = cpool.tile([128, NQ], F32, tag="gap4")
        nc.sync.dma_start(out=gap_sb[:, :], in_=gap4[:, :])
        id_sb = cpool.tile([B, B], F32, tag="ident")
        nc.sync.dma_start(out=id_sb[:, :], in_=ident[:, :])

        # ---- im2col (DRAM -> SBUF, overlapping source windows) ----
        imc = [[None] * 3 for _ in range(BPC)]
        for b in range(BPC):
            for ci, (k0, nk) in enumerate(KCH):
                t = work.tile([nk * 5, T], F32, tag=f"imc{b}{ci}")
                src = _ap(seq[:], b * D * L + k0, [[1, nk], [L, D], [1, T]])
                nc.sync.dma_start(out=t[:, :], in_=src)
                imc[b][ci] = t

        # ---- conv + tokens + argmax ----
        jm4f = work.tile([128, NQ], F32, tag="jm4f")
        for b in range(BPC):
            tokp = tokps.tile([1, L], F32, tag="tokp")
            nc.tensor.matmul(tokp[:, 0:512], wtok_sb[0:5, 0:1], imc[b][0][0:5, 0:512])
            nc.tensor.matmul(tokp[:, 512:961], wtok_sb[0:5, 0:1],
                             imc[b][0][0:5, 512:961])
            # rows 64:70 of chunk 2 are (k=62,d=4),(k=63,d=0..4); col 1 of wtok
            # is shifted so the k=62 row gets weight 0.
            nc.tensor.matmul(tokp[:, 961:1024], wtok_sb[64:70, 1:2],
                             imc[b][2][64:70, 898:961])
            toks = spool.tile([1, L], F32, tag="toks")
            nc.vector.tensor_copy(toks[:, :], tokp[:, :])
            nc.sync.dma_start(out=tokd[b, :], in_=toks[0:1, :])

            for ch in range(2):
                cp = psum.tile([128, T], F32, tag="convp")
                for ci in range(3):
                    for (n0, nn) in NCH:
                        nc.tensor.matmul(
                            cp[:, n0:n0 + nn],
                            w2t_sb[ci][:, ch * 128:(ch + 1) * 128],
                            imc[b][ci][:, n0:n0 + nn],
                            start=(ci == 0), stop=(ci == 2))
                cs = spool.tile([128, T], F32, tag="convs")
                nc.vector.tensor_copy(cs[:, :], cp[:, :])
                mx8 = spool.tile([128, 8], F32, tag="mx8")
                nc.vector.max(mx8[:, :], cs[:, :])
                ix8 = spool.tile([128, 8], U16, tag="ix8")
                nc.vector.max_index(ix8[:, :], mx8[:, :], cs[:, :])
                q = ch * 2 + b
                nc.vector.tensor_copy(jm4f[:, q:q + 1], ix8[:, 0:1])

        # window start index = 1024*b_lo + jmin  (q = c_hi*2 + b_lo)
        nc.vector.tensor_scalar(jm4f[:, 1:4:2], jm4f[:, 1:4:2], 1024.0, None,
                                op0=ALU.add)

        # ---- expand token windows in DRAM (aligned rows for dma_gather) ----
        for b in range(BPC):
            nc.sync.dma_start(
                out=tokwin[b * L:b * L + T, :],
                in_=_ap(tokd[:], b * L, [[1, T], [1, K]]))
            # rows T..L-1 are never gathered (jmin <= T-1) but must be finite
            nc.sync.dma_start(
                out=tokwin[b * L + T:(b + 1) * L, :],
                in_=_ap(tokd[:], b * L, [[1, L - T], [1, K]]))

        # ---- bounce jmin through DRAM (p-major write, wrapped read) ----
        nc.sync.dma_start(out=_ap(jmd[:], 0, [[4, 128], [1, NQ]]), in_=jm4f[:, :])
        jmw = spool.tile([16, 8 * NQ], F32, tag="jmw")
        nc.sync.dma_start(out=jmw[:, :],
                          in_=_ap(jmd[:], 0, [[4, 16], [64, 8], [1, NQ]]))
        # idx1[pl, m1*8] = window row of problem (ph*16+pl, q), m1 = ph*4+q;
        # other columns are dummy index 0 (gathered into partitions 16..127).
        idxw = spool.tile([128, 8 * NQ * 8], I16, tag="idxw")
        nc.gpsimd.memset(idxw[:, :], 0)
        nc.vector.tensor_copy(idxw[0:16, 0:256:8], jmw[:, :])
        # HW gpsimd cores each read their own 16-partition idx block: replicate
        for k in range(1, 8):
            nc.sync.dma_start(out=idxw[16 * k:16 * (k + 1), :], in_=idxw[0:16, :])

        # ---- gather #1: token windows onto partitions 0..15 ----
        tokw = work.tile([128, 8 * NQ, K], F32, tag="tokw")
        for c in range(4):  # SWDGE fifo holds ~125*16 idx; chunk at 1024
            nc.gpsimd.dma_gather(
                out_ap=tokw[:, c * 8:(c + 1) * 8, :], in_ap=tokwin[:, :],
                idxs_ap=idxw[:, c * 64:(c + 1) * 64],
                num_idxs=1024, num_idxs_reg=1024, elem_size=K)

        nc.sync.dma_start(out=dbg_jm[:, :], in_=jm4f[:, :])
        nc.sync.dma_start(out=dbg_tokw[:, :],
                          in_=tokw[0:16, :, :].rearrange("p a b -> p (a b)"))
        # ---- pair indices on 16 partitions ----
        # pix16[pl, (ph,q), ip] = 25*(ph*16+pl) + 3200*c_hi + 5*t_2ip + t_2ip+1
        pix = work.tile([16, 8 * NQ, NPAIR], F32, tag="pix")
        nc.vector.scalar_tensor_tensor(
            pix[:, :, :], tokw[0:16, :, 0:K:2], 5.0, tokw[0:16, :, 1:K:2],
            op0=ALU.mult, op1=ALU.add)
        nc.vector.tensor_tensor(
            pix[:, :, :], pix[:, :, :],
            c25_sb[:, :].rearrange("p (m i) -> p m i", m=8 * NQ), ALU.add)
        # idx2[pl, ip*32 + q*8 + ph] = pix16[pl, (ph*4+q)*32 + ip], cast to i16
        idxp = work.tile([128, NPROB * NPAIR // 16], I16, tag="idxp")
        nc.vector.tensor_copy(
            _ap(idxp[:, :], 0, [[1024, 16], [32, NPAIR], [8, NQ], [1, 8]]),
            _ap(pix[:, :, :], 0, [[1024, 16], [1, NPAIR], [32, NQ], [128, 8]]))
        for k in range(1, 8):
            nc.sync.dma_start(out=idxp[16 * k:16 * (k + 1), :], in_=idxp[0:16, :])

        # ---- gather #2: s row-pairs [128, 32, 4, 128] (pre-divided by gap) ----
        s_sb = work.tile([128, NPAIR, NQ, 2 * K], F32, tag="s")
        s_flat = s_sb[:, :, :, :].rearrange("p a b c -> p (a b) c")
        for c in range(16):
            nc.gpsimd.dma_gather(
                out_ap=s_flat[:, c * 8:(c + 1) * 8, :], in_ap=wpair[:, :],
                idxs_ap=idxp[:, c * 64:(c + 1) * 64],
                num_idxs=1024, num_idxs_reg=1024, elem_size=2 * K)

        nc.sync.dma_start(out=dbg_pix[:, :],
                          in_=pix[:, :, :].rearrange("p a b -> p (a b)"))
        nc.sync.dma_start(out=dbg_s[:, :],
                          in_=s_sb[:, 0, :, :].rearrange("p a b -> p (a b)"))
        # ---- hard-NW DP over rows; Z[i,j] = D/gap + i + j (all boundaries 0) ----
        Z = work.tile([128, NQ, K + 1], F32, tag="Z")
        cb = work.tile([128, NQ, K], F32, tag="cb")
        nc.gpsimd.memset(Z[:, :, :], 0.0)
        for i in range(1, K + 1):
            ip, h = (i - 1) // 2, (i - 1) % 2
            nc.vector.scalar_tensor_tensor(
                cb[:, :, :], Z[:, :, 0:K], 2.0, Z[:, :, 1:K + 1],
                op0=ALU.add, op1=ALU.max)
            for q in range(NQ):
                nc.vector.tensor_tensor_scan(
                    Z[:, q, 1:K + 1], cb[:, q, :], s_sb[:, ip, q, h * K:(h + 1) * K],
                    0.0, op0=ALU.max, op1=ALU.add)

        # ---- embed = relu(gap*(Z64 - 128) + bias) ----
        sc = work.tile([128, NQ], F32, tag="sc")
        nc.vector.tensor_scalar(
            sc[:, :], Z[:, :, K:K + 1].rearrange("p q o -> p (q o)"),
            -128.0, None, op0=ALU.add)
        nc.vector.tensor_tensor(sc[:, :], sc[:, :], gap_sb[:, :], ALU.mult)
        nc.vector.tensor_tensor(sc[:, :], sc[:, :], bias_sb[:, :], ALU.add)
        nc.vector.tensor_scalar(sc[:, :], sc[:, :], 0.0, None, op0=ALU.max)

        nc.sync.dma_start(out=dbg_sc[:, :], in_=sc[:, :])
        # ---- all-gather embeddings, Gram, normalize via diagonal ----
        # embd local layout: addr = c_hi*256 + p*2 + b_lo
        nc.sync.dma_start(out=_ap(embd[:], 0, [[2, 128], [256, 2], [1, 2]]),
                          in_=sc[:, :])
        nc.gpsimd.collective_compute(
            "AllGather", ALU.bypass, replica_groups=[list(range(NCORES))],
            ins=[embd[:]], outs=[emba[:]])

        nc.sync.dma_start(out=dbg_emb[:, :], in_=emba[:, :])
        nc.sync.dma_start(out=dbg_tok[0, :], in_=_ap(tokd[:], 0, [[1, 1], [1, L]]))
        nc.sync.dma_start(out=dbg_tok[1, :], in_=_ap(tokd[:], L, [[1, 1], [1, L]]))
        et = work.tile([128, 2, B], F32, tag="et")
        for r in range(NCORES):
            nc.sync.dma_start(
                out=et[:, :, r * 2:(r + 1) * 2],
                in_=_ap(emba[:], r * 512, [[2, 128], [256, 2], [1, 2]]))
        G = gpsum.tile([B, B], F32, tag="gram")
        for hh in range(2):
            nc.tensor.matmul(G[:, :], et[:, hh, :], et[:, hh, :],
                             start=(hh == 0), stop=(hh == 1))

        dm = spool.tile([B, B], F32, tag="dm")
        nc.vector.tensor_tensor(dm[:, :], G[:, :], id_sb[:, :], ALU.mult)
        d16 = spool.tile([B, 1], F32, tag="d16")
        nc.vector.tensor_reduce(d16[:, :], dm[:, :], mybir.AxisListType.X, ALU.add)
        dinv = spool.tile([B, 1], F32, tag="dinv")
        nc.vector.reciprocal(dinv[:, :], d16[:, :])
        r = spool.tile([B, 1], F32, tag="r")
        nc.scalar.activation(r[:, :], dinv[:, :],
                             mybir.ActivationFunctionType.Sqrt)
        t1 = spool.tile([B, 1], F32, tag="t1")
        for _ in range(2):  # Newton: r <- r*(1.5 - 0.5*d*r^2)
            nc.vector.tensor_tensor(t1[:, :], r[:, :], r[:, :], ALU.mult)
            nc.vector.tensor_tensor(t1[:, :], t1[:, :], d16[:, :], ALU.mult)
            nc.vector.tensor_scalar(t1[:, :], t1[:, :], -0.5, 1.5,
                                    op0=ALU.mult, op1=ALU.add)
            nc.vector.tensor_tensor(r[:, :], r[:, :], t1[:, :], ALU.mult)

        H = spool.tile([B, B], F32, tag="H")
        nc.vector.tensor_scalar(H[:, :], G[:, :], r[:, 0:1], None, op0=ALU.mult)
        Ht = gpsum.tile([B, B], F32, tag="Ht")
        nc.tensor.transpose(Ht[:, :], H[:, :], id_sb[:, :])
        Fo = spool.tile([B, B], F32, tag="Fo")
        nc.vector.tensor_scalar(Fo[:, :], Ht[:, :], r[:, 0:1], None, op0=ALU.mult)
        nc.vector.tensor_scalar(Fo[:, :], Fo[:, :], -1.0, 1.0,
                                op0=ALU.mult, op1=ALU.add)
        nc.sync.dma_start(out=outx[:, :], in_=Fo[:, :])

    return nc


def _graph():
    global _GRAPH
    if _GRAPH is None:
        g = _build()
        g.finalize()
        _GRAPH = g
    return _GRAPH


def _host_inputs(seq_oh, weights, bias, gap):
    W = np.asarray(weights, np.float32)
    g = np.asarray(gap, np.float32)
    bias = np.asarray(bias, np.float32)
    seq_oh = np.asarray(seq_oh, np.float32)

    w2t = np.ascontiguousarray(W.transpose(2, 1, 0).reshape(KD, C))
    wtok = np.zeros((128, 2), np.float32)
    wtok[0:5, 0] = np.arange(D)
    wtok[65:70, 1] = np.arange(D)
    Wg = W / g[:, None, None]
    wp = np.empty((C, D, D, 2 * K), np.float32)
    wp[:, :, :, :K] = Wg[:, :, None, :]
    wp[:, :, :, K:] = Wg[:, None, :, :]
    wpair = wp.reshape(C * 25, 2 * K)
    # q = c_hi*2 + b_lo; pair-index table on 16 partitions:
    # c25[pl, (ph, q), ip] = 25*(ph*16+pl) + 3200*c_hi(q)
    pl = np.arange(16)
    ph = np.arange(8)
    qv = np.arange(NQ)
    c25 = (25.0 * (ph[None, :, None, None] * 16 + pl[:, None, None, None])
           + 3200.0 * (qv[None, None, :, None] // 2)
           + np.zeros((1, 1, 1, NPAIR))).astype(np.float32).reshape(16, -1)
    p_idx = np.arange(128, dtype=np.int64)
    cidx = (np.tile([0, 0, 128, 128], (128, 1)) + p_idx[:, None])
    bias4 = bias[cidx].astype(np.float32)
    gap4 = g[cidx].astype(np.float32)
    ident = np.eye(B, dtype=np.float32)

    common = dict(w2t=w2t, wtok=wtok, wpair=np.ascontiguousarray(wpair),
                  c25=np.ascontiguousarray(c25), bias4=bias4, gap4=gap4,
                  ident16=ident)
    in_maps = []
    for rcore in range(NCORES):
        m = dict(common)
        m["seq"] = np.ascontiguousarray(seq_oh[rcore * BPC:(rcore + 1) * BPC])
        in_maps.append(m)
    return in_maps


_LAST_RESULTS = None


def kernel(seq_oh, weights, bias, gap):
    global _LAST_RESULTS
    nc = _graph()
    in_maps = _host_inputs(seq_oh, weights, bias, gap)
    res = run_bass_kernel_spmd(nc, in_maps, core_ids=list(range(NCORES)))
    _LAST_RESULTS = res
    return np.asarray(res.results[0]["out"], np.float32)


def run_sim(seq_oh, weights, bias, gap, num_workers=8):
    """CoreSim-based check (no hardware). Returns (out, modeled_time_ns)."""
    from concourse import bass_interp
    nc = _graph()
    in_maps = _host_inputs(seq_oh, weights, bias, gap)
    sim = bass_interp.MultiCoreSim(nc, NCORES, num_workers=num_workers)
    for i in range(NCORES):
        for k, v in in_maps[i].items():
            sim.cores[i].tensor(k)[:] = v
    sim.simulate()
    t = getattr(sim, "global_time", None)
    return np.array(sim.cores[0].tensor("out"), np.float32), t


# revision 25
# speedup vs baseline: 1.1965x; 1.1965x over previous
"""Trainium2 Bass kernel for nn_AsMac: soft-NW motif embedding distance matrix.

Pipeline per core (batch-sharded, 2 sequences/core, all 256 channels):
  1. conv[b,c,t] via PE matmul on im2col (contraction 320 = K*D), fp32r
  2. token stream via tiny matmul row (one-hot dot [0..4])
  3. argmax_t conv -> jmin  (DVE max/max_index)
  4. gather token windows (dma_gather, aligned expanded rows)
  5. gather s-matrix row PAIRS straight from a host-prebuilt W-pair table
     (s[i,:] = W[c, tok_i, :] because seq is one-hot) -- no per-problem matmuls
  6. hard-NW DP (gamma=0.01 makes soft~hard; validated 4.7e-4 final rel err)
     row recurrence as ONE scalar_tensor_tensor + 4 tensor_tensor_scan per row
     over 512 problems [128 partitions x 4 blocks], in a shifted variable
     Z[i,j] = D/gap + i + j so all boundaries are 0 and no per-row constants
  7. embed = relu(gap*(Z[64,64]-128) + bias); AllGather embeds; Gram on PE;
     normalize via Gram diagonal rsqrt (Newton-refined); out = 1 - cos.
"""

import sys

for _p in ("/opt/trn_rl_repo",):
    if _p not in sys.path:
        sys.path.insert(0, _p)

import numpy as np
from contextlib import ExitStack

import concourse.bass as bass
import concourse.bacc as bacc
import concourse.tile as tile
from concourse import mybir
from concourse.bass_utils import run_bass_kernel_spmd

F32 = mybir.dt.float32
I16 = mybir.dt.int16
U16 = mybir.dt.uint16
ALU = mybir.AluOpType

B, D, L, C, K = 16, 5, 1024, 256, 64
NCORES = 8
BPC = B // NCORES            # 2 sequences per core
T = L - K + 1                # 961 conv positions
KD = K * D                   # 320 contraction rows (k-major, d-minor)
NQ = 4                       # blocks q = b_lo*2 + c_hi
NPAIR = K // 2               # 32 row-pairs per problem
NPROB = 128 * NQ             # 512 problems per core
KCH = [(0, 25), (25, 25), (50, 14)]      # k-chunks -> 125/125/70 contraction rows
NCH = [(0, 512), (512, 449)]             # conv N chunks (<=512 moving free dim)

_GRAPH = {}


def _ap(handle_ap, offset, dims):
    """Raw AP over the tensor underlying `handle_ap` (an AP or handle)."""
    t = handle_ap.tensor if isinstance(handle_ap, bass.AP) else handle_ap
    return bass.AP(t, offset, [list(d) for d in dims])


def _build(debug=False):
    nc = bacc.Bacc(num_devices=NCORES)

    seq = nc.declare_dram_parameter("seq", [BPC, D, L], F32, isOutput=False)
    w2t = nc.declare_dram_parameter("w2t", [KD, C], F32, isOutput=False)
    wtok = nc.declare_dram_parameter("wtok", [128, 2], F32, isOutput=False)
    wpair = nc.declare_dram_parameter("wpair", [C * 25, 2 * K], F32, isOutput=False)
    c25 = nc.declare_dram_parameter("c25", [128, 8 * NQ * NPAIR], F32, isOutput=False)
    bias4 = nc.declare_dram_parameter("bias4", [128, NQ], F32, isOutput=False)
    gap4 = nc.declare_dram_parameter("gap4", [128, NQ], F32, isOutput=False)
    ident = nc.declare_dram_parameter("ident16", [B, B], F32, isOutput=False)
    outx = nc.declare_dram_parameter("out", [B, B], F32, isOutput=True)
    if debug:
        dbg_jm = nc.declare_dram_parameter("dbg_jm", [128, NQ], F32, isOutput=True)
        dbg_tokw = nc.declare_dram_parameter("dbg_tokw", [16, 32 * K], F32,
                                             isOutput=True)
        dbg_pix = nc.declare_dram_parameter("dbg_pix", [128, 8 * NQ * NPAIR], F32,
                                            isOutput=True)
        dbg_s = nc.declare_dram_parameter("dbg_s", [128, NQ * 2 * K], F32,
                                          isOutput=True)
        dbg_sc = nc.declare_dram_parameter("dbg_sc", [128, NQ], F32, isOutput=True)
        dbg_emb = nc.declare_dram_parameter("dbg_emb", [B, C], F32, isOutput=True)
        dbg_tok = nc.declare_dram_parameter("dbg_tok", [BPC, L], F32, isOutput=True)

    tokd = nc.dram_tensor("tokd", [BPC, L], F32)
    tokwin = nc.dram_tensor("tokwin", [BPC * L, K], F32)
    jmd = nc.dram_tensor("jmd", [NPROB], F32)
    embd = nc.dram_tensor("embd", [BPC, C], F32)
    emba = nc.dram_tensor("emba", [B, C], F32, addr_space="Shared")

    with tile.TileContext(nc) as tc, ExitStack() as ctx:
        cpool = ctx.enter_context(tc.tile_pool(name="consts", bufs=1))
        work = ctx.enter_context(tc.tile_pool(name="work", bufs=1))
        spool = ctx.enter_context(tc.tile_pool(name="small", bufs=2))
        psum = ctx.enter_context(tc.tile_pool(name="psum", bufs=2, space="PSUM"))
        tokps = ctx.enter_context(tc.tile_pool(name="tokps", bufs=1, space="PSUM"))
        gpsum = ctx.enter_context(tc.tile_pool(name="gpsum", bufs=1, space="PSUM"))

        # ---- constants to SBUF ----
        w2t_sb = []
        for ci, (k0, nk) in enumerate(KCH):
            t = cpool.tile([nk * 5, C], F32, tag=f"w2t{ci}")
            nc.sync.dma_start(out=t[:, :], in_=w2t[k0 * 5:(k0 + nk) * 5, :])
            w2t_sb.append(t)
        wtok_sb = cpool.tile([128, 2], F32, tag="wtok")
        nc.sync.dma_start(out=wtok_sb[:, :], in_=wtok[:, :])
        c25_sb = cpool.tile([128, 8 * NQ * NPAIR], F32, tag="c25")
        nc.sync.dma_start(out=c25_sb[:, :], in_=c25[:, :])
        bias_sb = cpool.tile([128, NQ], F32, tag="bias4")
        nc.sync.dma_start(out=bias_sb[:, :], in_=bias4[:, :])
        gap_sb = cpool.tile([128, NQ], F32, tag="gap4")
        nc.sync.dma_start(out=gap_sb[:, :], in_=gap4[:, :])
        id_sb = cpool.tile([B, B], F32, tag="ident")
        nc.sync.dma_start(out=id_sb[:, :], in_=ident[:, :])

        # ---- im2col (DRAM -> SBUF, overlapping source windows) ----
        imc = [[None] * 3 for _ in range(BPC)]
        for b in range(BPC):
            for ci, (k0, nk) in enumerate(KCH):
                t = work.tile([nk * 5, T], F32, tag=f"imc{b}{ci}")
                src = _ap(seq[:], b * D * L + k0, [[1, nk], [L, D], [1, T]])
                nc.sync.dma_start(out=t[:, :], in_=src)
                imc[b][ci] = t

        # ---- tokens first (cheap, unblocks the gather pipeline early) ----
        for b in range(BPC):
            tokp = tokps.tile([1, L], F32, tag="tokp")
            nc.tensor.matmul(tokp[:, 0:512], wtok_sb[0:5, 0:1], imc[b][0][0:5, 0:512])
            nc.tensor.matmul(tokp[:, 512:961], wtok_sb[0:5, 0:1],
                             imc[b][0][0:5, 512:961])
            # rows 64:70 of chunk 2 are (k=62,d=4),(k=63,d=0..4); col 1 of wtok
            # is shifted so the k=62 row gets weight 0.
            nc.tensor.matmul(tokp[:, 961:1024], wtok_sb[64:70, 1:2],
                             imc[b][2][64:70, 898:961])
            toks = spool.tile([1, L], F32, tag="toks")
            nc.scalar.copy(toks[:, :], tokp[:, :])
            nc.sync.dma_start(out=tokd[b, :], in_=toks[0:1, :])
            # expand token windows in DRAM (aligned rows for dma_gather);
            # rows T..L-1 are never gathered (jmin <= T-1) but must be finite
            nc.sync.dma_start(
                out=tokwin[b * L:b * L + T, :],
                in_=_ap(tokd[:], b * L, [[1, T], [1, K]]))
            nc.sync.dma_start(
                out=tokwin[b * L + T:(b + 1) * L, :],
                in_=_ap(tokd[:], b * L, [[1, L - T], [1, K]]))

        # ---- conv + argmax ----
        jm4f = work.tile([128, NQ], F32, tag="jm4f")
        for b in range(BPC):
            for ch in range(2):
                cp = psum.tile([128, T], F32, tag="convp")
                for ci in range(3):
                    for (n0, nn) in NCH:
                        nc.tensor.matmul(
                            cp[:, n0:n0 + nn],
                            w2t_sb[ci][:, ch * 128:(ch + 1) * 128]
                            .bitcast(mybir.dt.float32r),
                            imc[b][ci][:, n0:n0 + nn]
                            .bitcast(mybir.dt.float32r),
                            start=(ci == 0), stop=(ci == 2))
                cs = spool.tile([128, T], F32, tag="convs")
                nc.scalar.copy(cs[:, :], cp[:, :])
                mx8 = spool.tile([128, 8], F32, tag="mx8")
                nc.vector.max(mx8[:, :], cs[:, :])
                ix8 = spool.tile([128, 8], U16, tag="ix8")
                nc.vector.max_index(ix8[:, :], mx8[:, :], cs[:, :])
                q = ch * 2 + b
                nc.vector.tensor_copy(jm4f[:, q:q + 1], ix8[:, 0:1])

        # window start index = 1024*b_lo + jmin  (q = c_hi*2 + b_lo)
        nc.vector.tensor_scalar(jm4f[:, 1:4:2], jm4f[:, 1:4:2], 1024.0, None,
                                op0=ALU.add)

        # ---- bounce jmin through DRAM (p-major write, wrapped read) ----
        nc.sync.dma_start(out=_ap(jmd[:], 0, [[4, 128], [1, NQ]]), in_=jm4f[:, :])
        jmw = spool.tile([16, 8 * NQ], F32, tag="jmw")
        nc.sync.dma_start(out=jmw[:, :],
                          in_=_ap(jmd[:], 0, [[4, 16], [64, 8], [1, NQ]]))
        # idx1[pl, m1*8 + ph'] = window row of problem (ph*16+pl, q): replicate
        # into all 8 slots so gathered windows land replicated on all 128
        # partitions (lets the pair-index math run on 128 partitions).
        idxw = spool.tile([128, 8 * NQ * 8], I16, tag="idxw")
        for ph2 in range(8):
            nc.vector.tensor_copy(idxw[0:16, ph2:256:8], jmw[:, :])
        # HW gpsimd cores each read their own 16-partition idx block: replicate
        for k in range(1, 8):
            nc.sync.dma_start(out=idxw[16 * k:16 * (k + 1), :], in_=idxw[0:16, :])

        # ---- gather #1: token windows onto partitions 0..15 ----
        tokw = work.tile([128, 8 * NQ, K], F32, tag="tokw")
        for c in range(4):  # SWDGE fifo holds ~125*16 idx; chunk at 1024
            nc.gpsimd.dma_gather(
                out_ap=tokw[:, c * 8:(c + 1) * 8, :], in_ap=tokwin[:, :],
                idxs_ap=idxw[:, c * 64:(c + 1) * 64],
                num_idxs=1024, num_idxs_reg=1024, elem_size=K)

        if debug:
            nc.sync.dma_start(out=dbg_jm[:, :], in_=jm4f[:, :])
            nc.sync.dma_start(out=dbg_tokw[:, :],
                              in_=tokw[0:16, :, :].rearrange("p a b -> p (a b)"))
        # ---- pair indices, computed replicated on all 128 partitions ----
        # pix[p, (ph,q), ip] = 5*t_2ip + t_2ip+1 (+ 25*(ph*16+p%16) + 3200*c_hi
        # merged into the cast below via c25)
        pix = work.tile([128, 8 * NQ, NPAIR], F32, tag="pix")
        nc.vector.scalar_tensor_tensor(
            pix[:, :, :], tokw[:, :, 0:K:2], 5.0, tokw[:, :, 1:K:2],
            op0=ALU.mult, op1=ALU.add)
        # idx2[p, ip*32 + q*8 + ph] = pix[p, (ph*4+q)*32 + ip] + c25, cast i16
        idxp = work.tile([128, NPROB * NPAIR // 16], I16, tag="idxp")
        nc.vector.tensor_tensor(
            _ap(idxp[:, :], 0, [[1024, 128], [32, NPAIR], [8, NQ], [1, 8]]),
            _ap(pix[:, :, :], 0, [[1024, 128], [1, NPAIR], [32, NQ], [128, 8]]),
            _ap(c25_sb[:, :], 0, [[1024, 128], [1, NPAIR], [32, NQ], [128, 8]]),
            ALU.add)

        # ---- gather #2: s row-pairs [128, 32, 4, 128] (pre-divided by gap) ----
        s_sb = work.tile([128, NPAIR, NQ, 2 * K], F32, tag="s")
        s_flat = s_sb[:, :, :, :].rearrange("p a b c -> p (a b) c")
        for c in range(16):
            nc.gpsimd.dma_gather(
                out_ap=s_flat[:, c * 8:(c + 1) * 8, :], in_ap=wpair[:, :],
                idxs_ap=idxp[:, c * 64:(c + 1) * 64],
                num_idxs=1024, num_idxs_reg=1024, elem_size=2 * K)

        if debug:
            nc.sync.dma_start(out=dbg_pix[:, :],
                              in_=pix[:, :, :].rearrange("p a b -> p (a b)"))
            nc.sync.dma_start(out=dbg_s[:, :],
                              in_=s_sb[:, 0, :, :].rearrange("p a b -> p (a b)"))
        # ---- hard-NW DP over rows; Z[i,j] = D/gap + i + j (all boundaries 0) ----
        Z = work.tile([128, NQ, K + 1], F32, tag="Z")
        cb = work.tile([128, NQ, K], F32, tag="cb")
        nc.gpsimd.memset(Z[:, :, :], 0.0)
        for i in range(1, K + 1):
            ip, h = (i - 1) // 2, (i - 1) % 2
            nc.vector.scalar_tensor_tensor(
                cb[:, :, :], Z[:, :, 0:K], 2.0, Z[:, :, 1:K + 1],
                op0=ALU.add, op1=ALU.max)
            for q in range(NQ):
                nc.vector.tensor_tensor_scan(
                    Z[:, q, 1:K + 1], cb[:, q, :], s_sb[:, ip, q, h * K:(h + 1) * K],
                    0.0, op0=ALU.max, op1=ALU.add)

        # ---- embed = relu(gap*(Z64 - 128) + bias) ----
        sc = work.tile([128, NQ], F32, tag="sc")
        nc.vector.tensor_scalar(
            sc[:, :], Z[:, :, K:K + 1].rearrange("p q o -> p (q o)"),
            -128.0, None, op0=ALU.add)
        nc.vector.tensor_tensor(sc[:, :], sc[:, :], gap_sb[:, :], ALU.mult)
        nc.vector.tensor_tensor(sc[:, :], sc[:, :], bias_sb[:, :], ALU.add)
        nc.vector.tensor_scalar(sc[:, :], sc[:, :], 0.0, None, op0=ALU.max)

        if debug:
            nc.sync.dma_start(out=dbg_sc[:, :], in_=sc[:, :])
        # ---- all-gather embeddings, Gram, normalize via diagonal ----
        # embd local layout: addr = c_hi*256 + p*2 + b_lo
        nc.sync.dma_start(out=_ap(embd[:], 0, [[2, 128], [256, 2], [1, 2]]),
                          in_=sc[:, :])
        nc.gpsimd.collective_compute(
            "AllGather", ALU.bypass, replica_groups=[list(range(NCORES))],
            ins=[embd[:]], outs=[emba[:]])

        if debug:
            nc.sync.dma_start(out=dbg_emb[:, :], in_=emba[:, :])
            nc.sync.dma_start(out=dbg_tok[0, :],
                              in_=_ap(tokd[:], 0, [[1, 1], [1, L]]))
            nc.sync.dma_start(out=dbg_tok[1, :],
                              in_=_ap(tokd[:], L, [[1, 1], [1, L]]))
        et = work.tile([128, 2, B], F32, tag="et")
        for r in range(NCORES):
            nc.sync.dma_start(
                out=et[:, :, r * 2:(r + 1) * 2],
                in_=_ap(emba[:], r * 512, [[2, 128], [256, 2], [1, 2]]))
        G = gpsum.tile([B, B], F32, tag="gram")
        for hh in range(2):
            nc.tensor.matmul(G[:, :], et[:, hh, :], et[:, hh, :],
                             start=(hh == 0), stop=(hh == 1))

        dm = spool.tile([B, B], F32, tag="dm")
        nc.vector.tensor_tensor(dm[:, :], G[:, :], id_sb[:, :], ALU.mult)
        d16 = spool.tile([B, 1], F32, tag="d16")
        nc.vector.tensor_reduce(d16[:, :], dm[:, :], mybir.AxisListType.X, ALU.add)
        dinv = spool.tile([B, 1], F32, tag="dinv")
        nc.vector.reciprocal(dinv[:, :], d16[:, :])
        r = spool.tile([B, 1], F32, tag="r")
        nc.scalar.activation(r[:, :], dinv[:, :],
                             mybir.ActivationFunctionType.Sqrt)
        t1 = spool.tile([B, 1], F32, tag="t1")
        for _ in range(2):  # Newton: r <- r*(1.5 - 0.5*d*r^2)
            nc.vector.tensor_tensor(t1[:, :], r[:, :], r[:, :], ALU.mult)
            nc.vector.tensor_tensor(t1[:, :], t1[:, :], d16[:, :], ALU.mult)
            nc.vector.tensor_scalar(t1[:, :], t1[:, :], -0.5, 1.5,
                                    op0=ALU.mult, op1=ALU.add)
            nc.vector.tensor_tensor(r[:, :], r[:, :], t1[:, :], ALU.mult)

        H = spool.tile([B, B], F32, tag="H")
        nc.vector.tensor_scalar(H[:, :], G[:, :], r[:, 0:1], None, op0=ALU.mult)
        Ht = gpsum.tile([B, B], F32, tag="Ht")
        nc.tensor.transpose(Ht[:, :], H[:, :], id_sb[:, :])
        Fo = spool.tile([B, B], F32, tag="Fo")
        nc.vector.tensor_scalar(Fo[:, :], Ht[:, :], r[:, 0:1], None, op0=ALU.mult)
        nc.vector.tensor_scalar(Fo[:, :], Fo[:, :], -1.0, 1.0,
                                op0=ALU.mult, op1=ALU.add)
        nc.sync.dma_start(out=outx[:, :], in_=Fo[:, :])

    return nc


def _graph():
    return _graph_for(False)


def _graph_for(debug):
    if debug not in _GRAPH:
        g = _build(debug=debug)
        g.finalize()
        _GRAPH[debug] = g
    return _GRAPH[debug]


def _host_inputs(seq_oh, weights, bias, gap):
    W = np.asarray(weights, np.float32)
    g = np.asarray(gap, np.float32)
    bias = np.asarray(bias, np.float32)
    seq_oh = np.asarray(seq_oh, np.float32)

    w2t = np.ascontiguousarray(W.transpose(2, 1, 0).reshape(KD, C))
    wtok = np.zeros((128, 2), np.float32)
    wtok[0:5, 0] = np.arange(D)
    wtok[65:70, 1] = np.arange(D)
    Wg = W / g[:, None, None]
    wp = np.empty((C, D, D, 2 * K), np.float32)
    wp[:, :, :, :K] = Wg[:, :, None, :]
    wp[:, :, :, K:] = Wg[:, None, :, :]
    wpair = wp.reshape(C * 25, 2 * K)
    # q = c_hi*2 + b_lo; pair-index table on 16 partitions:
    # c25[pl, (ph, q), ip] = 25*(ph*16+pl) + 3200*c_hi(q)
    pv = np.arange(128)
    ph = np.arange(8)
    qv = np.arange(NQ)
    c25 = (25.0 * (ph[None, :, None, None] * 16 + (pv % 16)[:, None, None, None])
           + 3200.0 * (qv[None, None, :, None] // 2)
           + np.zeros((1, 1, 1, NPAIR))).astype(np.float32).reshape(128, -1)
    p_idx = np.arange(128, dtype=np.int64)
    cidx = (np.tile([0, 0, 128, 128], (128, 1)) + p_idx[:, None])
    bias4 = bias[cidx].astype(np.float32)
    gap4 = g[cidx].astype(np.float32)
    ident = np.eye(B, dtype=np.float32)

    common = dict(w2t=w2t, wtok=wtok, wpair=np.ascontiguousarray(wpair),
                  c25=np.ascontiguousarray(c25), bias4=bias4, gap4=gap4,
                  ident16=ident)
    in_maps = []
    for rcore in range(NCORES):
        m = dict(common)
        m["seq"] = np.ascontiguousarray(seq_oh[rcore * BPC:(rcore + 1) * BPC])
        in_maps.append(m)
    return in_maps


_LAST_RESULTS = None


def kernel(seq_oh, weights, bias, gap):
    global _LAST_RESULTS
    nc = _graph()
    in_maps = _host_inputs(seq_oh, weights, bias, gap)
    res = run_bass_kernel_spmd(nc, in_maps, core_ids=list(range(NCORES)))
    _LAST_RESULTS = res
    return np.asarray(res.results[0]["out"], np.float32)


def run_sim(seq_oh, weights, bias, gap, num_workers=8, trace=False):
    """CoreSim-based check (no hardware). Returns (out, modeled_time_ns)."""
    from concourse import bass_interp
    nc = _graph()
    in_maps = _host_inputs(seq_oh, weights, bias, gap)
    sim = bass_interp.MultiCoreSim(nc, NCORES, num_workers=num_workers, trace=trace)
    for i in range(NCORES):
        for k, v in in_maps[i].items():
            sim.cores[i].tensor(k)[:] = v
    sim.simulate()
    t = getattr(sim, "global_time", None)
    return np.array(sim.cores[0].tensor("out"), np.float32), t


# revision 27
# speedup vs baseline: 1.3162x; 1.1000x over previous
"""Trainium2 Bass kernel for nn_AsMac: soft-NW motif embedding distance matrix.

Pipeline per core (batch-sharded, 2 sequences/core, all 256 channels):
  1. conv[b,c,t] via PE matmul on im2col (contraction 320 = K*D), float32r
  2. token stream via tiny matmul row (one-hot dot [0..4])
  3. argmax_t conv -> jmin  (DVE MAX8/FIND_INDEX_8, SBUF source)
  4. dma_gather token windows (aligned expanded rows), replicated onto all
     128 partitions (the gpsimd cores each read their own 16-partition idx
     block, so indices are replicated 8x)
  5. dma_gather s-matrix CHANNEL-PAIR rows from a host-prebuilt table
     (s[i,:] = W[c, tok_i, :]/gap[c] since seq is one-hot): element m=i*2+w
     holds [row(c=p, v=2w) | row(c=128+p, v=2w+1)] so each DP row reads a
     contiguous [128, 256] slice covering all 4 problem blocks
  6. hard-NW DP (gamma=0.01 makes soft~hard; validated 4.7e-4 final rel err)
     row recurrence: tiny TT (j=1 boundary) + scalar_tensor_tensor + ONE
     merged tensor_tensor_scan over all 4 blocks [128, 256] per row.
     Blocks are kept in shifted spaces Z_v = D/gap + i + j + 256*v; the
     ascending 256*v offsets make cross-block scan-state leakage harmless.
  7. embed = relu(gap*(Z-128-256v) + bias); AllGather embeds; Gram on PE
     (full fp32); normalize via Gram diagonal rsqrt (Newton-refined);
     out = 1 - cosine similarity.

Problem slots per core: partition p, block v = b_lo*2 + c_hi
  -> sequence b = 2*core + b_lo, channel c = c_hi*128 + p.
"""

import sys

for _p in ("/opt/trn_rl_repo",):
    if _p not in sys.path:
        sys.path.insert(0, _p)

import numpy as np
from contextlib import ExitStack

import concourse.bass as bass
import concourse.bacc as bacc
import concourse.tile as tile
from concourse import mybir
from concourse.bass_utils import run_bass_kernel_spmd

F32 = mybir.dt.float32
I16 = mybir.dt.int16
U16 = mybir.dt.uint16
ALU = mybir.AluOpType

B, D, L, C, K = 16, 5, 1024, 256, 64
NCORES = 8
BPC = B // NCORES            # 2 sequences per core
T = L - K + 1                # 961 conv positions
KD = K * D                   # 320 contraction rows (k-major, d-minor)
NQ = 4                       # blocks v = b_lo*2 + c_hi
NPROB = 128 * NQ             # 512 problems per core
BV = 256.0                   # per-block shift in the merged scan
KCH = [(0, 25), (25, 25), (50, 14)]      # k-chunks -> 125/125/70 contraction rows
NCH = [(0, 512), (512, 449)]             # conv N chunks (<=512 moving free dim)

_GRAPH = {}


def _ap(handle_ap, offset, dims):
    """Raw AP over the tensor underlying `handle_ap` (an AP or handle)."""
    t = handle_ap.tensor if isinstance(handle_ap, bass.AP) else handle_ap
    return bass.AP(t, offset, [list(d) for d in dims])


def _build(debug=False):
    nc = bacc.Bacc(num_devices=NCORES)

    seq = nc.declare_dram_parameter("seq", [BPC, D, L], F32, isOutput=False)
    w2t = nc.declare_dram_parameter("w2t", [KD, C], F32, isOutput=False)
    wtok = nc.declare_dram_parameter("wtok", [128, 2], F32, isOutput=False)
    wpair = nc.declare_dram_parameter("wpair", [128 * 25, 2 * K], F32,
                                      isOutput=False)
    c25 = nc.declare_dram_parameter("c25", [128, 1024], F32, isOutput=False)
    bias4 = nc.declare_dram_parameter("bias4", [128, NQ], F32, isOutput=False)
    gap4 = nc.declare_dram_parameter("gap4", [128, NQ], F32, isOutput=False)
    cB = nc.declare_dram_parameter("cB", [128, NQ], F32, isOutput=False)
    bcol2 = nc.declare_dram_parameter("bcol2", [128, NQ], F32, isOutput=False)
    ident = nc.declare_dram_parameter("ident16", [B, B], F32, isOutput=False)
    outx = nc.declare_dram_parameter("out", [B, B], F32, isOutput=True)
    if debug:
        dbg_jm = nc.declare_dram_parameter("dbg_jm", [128, NQ], F32, isOutput=True)
        dbg_s = nc.declare_dram_parameter("dbg_s", [128, 2 * 2 * K], F32,
                                          isOutput=True)
        dbg_sc = nc.declare_dram_parameter("dbg_sc", [128, NQ], F32, isOutput=True)
        dbg_emb = nc.declare_dram_parameter("dbg_emb", [B, C], F32, isOutput=True)

    tokd = nc.dram_tensor("tokd", [BPC, L], F32)
    tokwin = nc.dram_tensor("tokwin", [BPC * L, K], F32)
    jmd = nc.dram_tensor("jmd", [NPROB], F32)
    embd = nc.dram_tensor("embd", [BPC, C], F32)
    emba = nc.dram_tensor("emba", [B, C], F32, addr_space="Shared")

    with tile.TileContext(nc) as tc, ExitStack() as ctx:
        cpool = ctx.enter_context(tc.tile_pool(name="consts", bufs=1))
        work = ctx.enter_context(tc.tile_pool(name="work", bufs=1))
        spool = ctx.enter_context(tc.tile_pool(name="small", bufs=2))
        psum = ctx.enter_context(tc.tile_pool(name="psum", bufs=2, space="PSUM"))
        tokps = ctx.enter_context(tc.tile_pool(name="tokps", bufs=1, space="PSUM"))
        gpsum = ctx.enter_context(tc.tile_pool(name="gpsum", bufs=1, space="PSUM"))

        # ---- constants to SBUF ----
        w2t_sb = []
        for ci, (k0, nk) in enumerate(KCH):
            t = cpool.tile([nk * 5, C], F32, tag=f"w2t{ci}")
            nc.sync.dma_start(out=t[:, :], in_=w2t[k0 * 5:(k0 + nk) * 5, :])
            w2t_sb.append(t)
        wtok_sb = cpool.tile([128, 2], F32, tag="wtok")
        nc.sync.dma_start(out=wtok_sb[:, :], in_=wtok[:, :])
        c25_sb = cpool.tile([128, 1024], F32, tag="c25")
        nc.sync.dma_start(out=c25_sb[:, :], in_=c25[:, :])
        bias_sb = cpool.tile([128, NQ], F32, tag="bias4")
        nc.sync.dma_start(out=bias_sb[:, :], in_=bias4[:, :])
        gap_sb = cpool.tile([128, NQ], F32, tag="gap4")
        nc.sync.dma_start(out=gap_sb[:, :], in_=gap4[:, :])
        cB_sb = cpool.tile([128, NQ], F32, tag="cB")
        nc.sync.dma_start(out=cB_sb[:, :], in_=cB[:, :])
        bcol_sb = cpool.tile([128, NQ], F32, tag="bcol2")
        nc.sync.dma_start(out=bcol_sb[:, :], in_=bcol2[:, :])
        id_sb = cpool.tile([B, B], F32, tag="ident")
        nc.sync.dma_start(out=id_sb[:, :], in_=ident[:, :])

        # ---- im2col (DRAM -> SBUF, overlapping source windows) ----
        imc = [[None] * 3 for _ in range(BPC)]
        for b in range(BPC):
            for ci, (k0, nk) in enumerate(KCH):
                t = work.tile([nk * 5, T], F32, tag=f"imc{b}{ci}")
                src = _ap(seq[:], b * D * L + k0, [[1, nk], [L, D], [1, T]])
                nc.sync.dma_start(out=t[:, :], in_=src)
                imc[b][ci] = t

        # ---- tokens first (cheap, unblocks the gather pipeline early) ----
        for b in range(BPC):
            tokp = tokps.tile([1, L], F32, tag="tokp")
            nc.tensor.matmul(tokp[:, 0:512], wtok_sb[0:5, 0:1], imc[b][0][0:5, 0:512])
            nc.tensor.matmul(tokp[:, 512:961], wtok_sb[0:5, 0:1],
                             imc[b][0][0:5, 512:961])
            # rows 64:70 of chunk 2 are (k=62,d=4),(k=63,d=0..4); col 1 of wtok
            # is shifted so the k=62 row gets weight 0.
            nc.tensor.matmul(tokp[:, 961:1024], wtok_sb[64:70, 1:2],
                             imc[b][2][64:70, 898:961])
            toks = spool.tile([1, L], F32, tag="toks")
            nc.scalar.copy(toks[:, :], tokp[:, :])
            nc.sync.dma_start(out=tokd[b, :], in_=toks[0:1, :])
            # expand token windows in DRAM (aligned rows for dma_gather);
            # rows T..L-1 are never gathered (jmin <= T-1) but must be finite
            nc.sync.dma_start(
                out=tokwin[b * L:b * L + T, :],
                in_=_ap(tokd[:], b * L, [[1, T], [1, K]]))
            nc.sync.dma_start(
                out=tokwin[b * L + T:(b + 1) * L, :],
                in_=_ap(tokd[:], b * L, [[1, L - T], [1, K]]))

        # ---- conv + argmax ----
        jm4f = work.tile([128, NQ], F32, tag="jm4f")
        for b in range(BPC):
            for ch in range(2):
                cp = psum.tile([128, T], F32, tag="convp")
                for ci in range(3):
                    for (n0, nn) in NCH:
                        nc.tensor.matmul(
                            cp[:, n0:n0 + nn],
                            w2t_sb[ci][:, ch * 128:(ch + 1) * 128]
                            .bitcast(mybir.dt.float32r),
                            imc[b][ci][:, n0:n0 + nn]
                            .bitcast(mybir.dt.float32r),
                            start=(ci == 0), stop=(ci == 2))
                cs = spool.tile([128, T], F32, tag="convs")
                nc.scalar.copy(cs[:, :], cp[:, :])
                mx8 = spool.tile([128, 8], F32, tag="mx8")
                nc.vector.max(mx8[:, :], cs[:, :])
                ix8 = spool.tile([128, 8], U16, tag="ix8")
                nc.vector.max_index(ix8[:, :], mx8[:, :], cs[:, :])
                v = b * 2 + ch
                nc.vector.tensor_copy(jm4f[:, v:v + 1], ix8[:, 0:1])

        # window start index = 1024*b_lo + jmin  (v = b_lo*2 + c_hi)
        nc.vector.tensor_scalar(jm4f[:, 2:4], jm4f[:, 2:4], 1024.0, None,
                                op0=ALU.add)

        # ---- bounce jmin through DRAM (p-major write, wrapped read) ----
        nc.sync.dma_start(out=_ap(jmd[:], 0, [[4, 128], [1, NQ]]), in_=jm4f[:, :])
        jmw = spool.tile([16, 8 * NQ], F32, tag="jmw")
        nc.sync.dma_start(out=jmw[:, :],
                          in_=_ap(jmd[:], 0, [[4, 16], [64, 8], [1, NQ]]))
        # idx1[pl, m1*8 + ph'] = window row of problem (ph*16+pl, v), m1=ph*4+v,
        # replicated into all 8 ph' slots so gathered windows land replicated
        # on all 128 partitions (pair-index math then runs on 128 partitions).
        idxw = spool.tile([128, 8 * NQ * 8], I16, tag="idxw")
        for ph2 in range(8):
            nc.vector.tensor_copy(idxw[0:16, ph2:256:8], jmw[:, :])
        # HW gpsimd cores each read their own 16-partition idx block: replicate
        for k in range(1, 8):
            nc.sync.dma_start(out=idxw[16 * k:16 * (k + 1), :], in_=idxw[0:16, :])

        # ---- gather #1: token windows, replicated on all partitions ----
        tokw = work.tile([128, 8 * NQ, K], F32, tag="tokw")
        for c in range(4):  # SWDGE fifo holds ~125*16 idx; chunk at 1024
            nc.gpsimd.dma_gather(
                out_ap=tokw[:, c * 8:(c + 1) * 8, :], in_ap=tokwin[:, :],
                idxs_ap=idxw[:, c * 64:(c + 1) * 64],
                num_idxs=1024, num_idxs_reg=1024, elem_size=K)

        if debug:
            nc.sync.dma_start(out=dbg_jm[:, :], in_=jm4f[:, :])
        # ---- channel-pair indices on all 128 partitions ----
        # pix[p, (ph, w), i] = 5*tok(v=2w, i) + tok(v=2w+1, i); then the
        # merged add+cast below adds 25*(ph*16 + p%16) and converts to i16.
        pix = work.tile([128, 8, 2, K], F32, tag="pix")
        nc.vector.scalar_tensor_tensor(
            _ap(pix[:, :, :, :], 0, [[1024, 128], [128, 8], [64, 2], [1, K]]),
            _ap(tokw[:, :, :], 0, [[2048, 128], [256, 8], [128, 2], [1, K]]),
            5.0,
            _ap(tokw[:, :, :], 64, [[2048, 128], [256, 8], [128, 2], [1, K]]),
            op0=ALU.mult, op1=ALU.add)
        # idx2[p, i*16 + w*8 + ph] = pix + c25, cast to i16
        idxp = work.tile([128, 1024], I16, tag="idxp")
        nc.vector.tensor_tensor(
            _ap(idxp[:, :], 0, [[1024, 128], [1, 8], [8, 2], [16, K]]),
            _ap(pix[:, :, :, :], 0, [[1024, 128], [128, 8], [64, 2], [1, K]]),
            _ap(c25_sb[:, :], 0, [[1024, 128], [128, 8], [64, 2], [1, K]]),
            ALU.add)

        # ---- gather #2: s channel-pair rows [128, 64i, 2w, 128] ----
        s_sb = work.tile([128, K, 2, 2 * K], F32, tag="s")
        s_flat = s_sb[:, :, :, :].rearrange("p a b c -> p (a b) c")
        for c in range(16):
            nc.gpsimd.dma_gather(
                out_ap=s_flat[:, c * 8:(c + 1) * 8, :], in_ap=wpair[:, :],
                idxs_ap=idxp[:, c * 64:(c + 1) * 64],
                num_idxs=1024, num_idxs_reg=1024, elem_size=2 * K)

        if debug:
            nc.sync.dma_start(out=dbg_s[:, :],
                              in_=s_sb[:, 0, :, :].rearrange("p a b -> p (a b)"))
        # ---- hard-NW DP; Z_v[i,j] = D/gap + i + j + 256*v, boundaries = 256*v;
        #      one merged scan per row over all 4 blocks ----
        Z = work.tile([128, NQ, K], F32, tag="Z")
        cb = work.tile([128, NQ, K], F32, tag="cb")
        for v in range(NQ):
            nc.gpsimd.memset(Z[:, v, :], BV * v)
        Zf = Z[:, :, :].rearrange("p a b -> p (a b)")
        cbf = cb[:, :, :].rearrange("p a b -> p (a b)")
        for i in range(1, K + 1):
            nc.vector.tensor_tensor(
                cb[:, :, 0:1],
                _ap(bcol_sb[:, :], 0, [[NQ, 128], [1, NQ], [1, 1]]),
                Z[:, :, 0:1], ALU.max)
            nc.vector.scalar_tensor_tensor(
                cb[:, :, 1:K], Z[:, :, 0:K - 1], 2.0, Z[:, :, 1:K],
                op0=ALU.add, op1=ALU.max)
            nc.vector.tensor_tensor_scan(
                Zf[:, :], cbf[:, :],
                s_sb[:, i - 1, :, :].rearrange("p a b -> p (a b)"),
                0.0, op0=ALU.max, op1=ALU.add)

        # ---- embed = relu(gap*(Z[63] - 128 - 256v) + bias), u = c_hi*2+b_lo ----
        sc = work.tile([128, NQ], F32, tag="sc")
        # iterate (c_hi, b_lo): Z slot v = b_lo*2 + c_hi -> strides (K, 2K)
        zlast = _ap(Z[:, :, :], K - 1, [[NQ * K, 128], [K, 2], [2 * K, 2]])
        cBu = _ap(cB_sb[:, :], 0, [[NQ, 128], [2, 2], [1, 2]])
        scu = _ap(sc[:, :], 0, [[NQ, 128], [2, 2], [1, 2]])
        nc.vector.tensor_tensor(scu, zlast, cBu, ALU.subtract)
        nc.vector.tensor_tensor(sc[:, :], sc[:, :], gap_sb[:, :], ALU.mult)
        nc.vector.tensor_tensor(sc[:, :], sc[:, :], bias_sb[:, :], ALU.add)
        nc.vector.tensor_scalar(sc[:, :], sc[:, :], 0.0, None, op0=ALU.max)

        if debug:
            nc.sync.dma_start(out=dbg_sc[:, :], in_=sc[:, :])
        # ---- all-gather embeddings, Gram, normalize via diagonal ----
        # sc slots u = c_hi*2 + b_lo; embd local layout addr = p*4 + u
        nc.sync.dma_start(out=_ap(embd[:], 0, [[4, 128], [1, NQ]]), in_=sc[:, :])
        nc.gpsimd.collective_compute(
            "AllGather", ALU.bypass, replica_groups=[list(range(NCORES))],
            ins=[embd[:]], outs=[emba[:]])

        if debug:
            nc.sync.dma_start(out=dbg_emb[:, :], in_=emba[:, :])
        # E^T loads: et[ch][c_lo, (r, b_lo)] = emba[r*512 + c_lo*4 + ch*2 + b_lo]
        et = [None, None]
        for chh in range(2):
            t = work.tile([128, B], F32, tag=f"et{chh}")
            nc.sync.dma_start(
                out=t[:, :],
                in_=_ap(emba[:], chh * 2, [[4, 128], [512, 8], [1, 2]]))
            et[chh] = t
        G = gpsum.tile([B, B], F32, tag="gram")
        for chh in range(2):
            nc.tensor.matmul(G[:, :], et[chh][:, :], et[chh][:, :],
                             start=(chh == 0), stop=(chh == 1))

        dm = spool.tile([B, B], F32, tag="dm")
        nc.vector.tensor_tensor(dm[:, :], G[:, :], id_sb[:, :], ALU.mult)
        d16 = spool.tile([B, 1], F32, tag="d16")
        nc.vector.tensor_reduce(d16[:, :], dm[:, :], mybir.AxisListType.X, ALU.add)
        dinv = spool.tile([B, 1], F32, tag="dinv")
        nc.vector.reciprocal(dinv[:, :], d16[:, :])
        r = spool.tile([B, 1], F32, tag="r")
        nc.scalar.activation(r[:, :], dinv[:, :],
                             mybir.ActivationFunctionType.Sqrt)
        t1 = spool.tile([B, 1], F32, tag="t1")
        for _ in range(2):  # Newton: r <- r*(1.5 - 0.5*d*r^2)
            nc.vector.tensor_tensor(t1[:, :], r[:, :], r[:, :], ALU.mult)
            nc.vector.tensor_tensor(t1[:, :], t1[:, :], d16[:, :], ALU.mult)
            nc.vector.tensor_scalar(t1[:, :], t1[:, :], -0.5, 1.5,
                                    op0=ALU.mult, op1=ALU.add)
            nc.vector.tensor_tensor(r[:, :], r[:, :], t1[:, :], ALU.mult)

        H = spool.tile([B, B], F32, tag="H")
        nc.vector.tensor_scalar(H[:, :], G[:, :], r[:, 0:1], None, op0=ALU.mult)
        Ht = gpsum.tile([B, B], F32, tag="Ht")
        nc.tensor.transpose(Ht[:, :], H[:, :], id_sb[:, :])
        Fo = spool.tile([B, B], F32, tag="Fo")
        nc.vector.tensor_scalar(Fo[:, :], Ht[:, :], r[:, 0:1], None, op0=ALU.mult)
        nc.vector.tensor_scalar(Fo[:, :], Fo[:, :], -1.0, 1.0,
                                op0=ALU.mult, op1=ALU.add)
        nc.sync.dma_start(out=outx[:, :], in_=Fo[:, :])

    return nc


def _graph():
    return _graph_for(False)


def _graph_for(debug):
    if debug not in _GRAPH:
        g = _build(debug=debug)
        g.finalize()
        _GRAPH[debug] = g
    return _GRAPH[debug]


def _host_inputs(seq_oh, weights, bias, gap):
    W = np.asarray(weights, np.float32)
    g = np.asarray(gap, np.float32)
    bias = np.asarray(bias, np.float32)
    seq_oh = np.asarray(seq_oh, np.float32)

    w2t = np.ascontiguousarray(W.transpose(2, 1, 0).reshape(KD, C))
    wtok = np.zeros((128, 2), np.float32)
    wtok[0:5, 0] = np.arange(D)
    wtok[65:70, 1] = np.arange(D)
    Wg = W / g[:, None, None]
    # channel-pair table: wpair[p*25 + t1*5 + t2] =
    #   [Wg[p, t1, :] | Wg[128+p, t2, :]]
    wp = np.empty((128, D, D, 2 * K), np.float32)
    wp[:, :, :, :K] = Wg[:128, :, None, :]
    wp[:, :, :, K:] = Wg[128:, None, :, :]
    wpair = wp.reshape(128 * 25, 2 * K)
    # c25[p, (ph, w, i)] = 25*(ph*16 + p%16)
    pv = np.arange(128)
    ph = np.arange(8)
    c25 = (25.0 * (ph[None, :, None, None] * 16 + (pv % 16)[:, None, None, None])
           + np.zeros((1, 1, 2, K))).astype(np.float32).reshape(128, 1024)
    # per-(p, u) tables, u = c_hi*2 + b_lo -> channel c = c_hi*128 + p
    cidx = (np.tile([0, 0, 128, 128], (128, 1)) + pv[:, None])
    bias4 = bias[cidx].astype(np.float32)
    gap4 = g[cidx].astype(np.float32)
    # cB[p, u] = 128 + BV*v(u), v = b_lo*2 + c_hi = (u&1)*2 + (u>>1)
    vu = np.array([(u % 2) * 2 + (u // 2) for u in range(NQ)])
    cB = np.tile(128.0 + BV * vu, (128, 1)).astype(np.float32)
    # bcol2[p, v] = BV*v + 2
    bcol2 = np.tile(BV * np.arange(NQ) + 2.0, (128, 1)).astype(np.float32)
    ident = np.eye(B, dtype=np.float32)

    common = dict(w2t=w2t, wtok=wtok, wpair=np.ascontiguousarray(wpair),
                  c25=np.ascontiguousarray(c25), bias4=bias4, gap4=gap4,
                  cB=cB, bcol2=bcol2, ident16=ident)
    in_maps = []
    for rcore in range(NCORES):
        m = dict(common)
        m["seq"] = np.ascontiguousarray(seq_oh[rcore * BPC:(rcore + 1) * BPC])
        in_maps.append(m)
    return in_maps


_LAST_RESULTS = None


def kernel(seq_oh, weights, bias, gap):
    global _LAST_RESULTS
    nc = _graph()
    in_maps = _host_inputs(seq_oh, weights, bias, gap)
    res = run_bass_kernel_spmd(nc, in_maps, core_ids=list(range(NCORES)))
    _LAST_RESULTS = res
    return np.asarray(res.results[0]["out"], np.float32)


def run_sim(seq_oh, weights, bias, gap, num_workers=8, trace=False, debug=False):
    """CoreSim-based check (no hardware). Returns (out, modeled_time_ns)."""
    from concourse import bass_interp
    nc = _graph_for(debug)
    in_maps = _host_inputs(seq_oh, weights, bias, gap)
    sim = bass_interp.MultiCoreSim(nc, NCORES, num_workers=num_workers,
                                   trace=trace)
    for i in range(NCORES):
        for k, v in in_maps[i].items():
            sim.cores[i].tensor(k)[:] = v
    sim.simulate()
    t = getattr(sim, "global_time", None)
    return np.array(sim.cores[0].tensor("out"), np.float32), t
